# revision 37
# baseline (speedup 1.0000x reference)
"""Trainium2 Bass kernel for BasicAttention (Bahdanau-style additive attention).

Math (per batch row b):
    target  = x @ W_in.T                                   [B, D]
    source  = context @ W_c.T + b_c                        [B, S, D]
    attn    = tanh(target[:, None, :] + source)            [B, S, D]
    scores  = attn @ w_v                                   [B, S]
    attn_w  = softmax(scores, axis=S)                      [B, S]
    weighted= attn_w @ context                             [B, D]
    h_tilde = tanh(concat([weighted, x], -1) @ W_out.T)    [B, D]

Sharding: data-parallel over batch. 8 cores x 4 batch rows, full weights
replicated per core, no collectives. Compute in bf16 (fp32 PSUM accumulation);
~0.2-0.5% of output scale vs the fp32 reference.

Per-core dataflow (all stationary matmul operands are SBUF-resident so their
LDWEIGHTS carry no semaphore waits and the PE's reorder window can pull them
ahead of in-flight matmuls):
  - weights are cast to bf16 via staged HWDGE loads + DVE copies, bounced
    through DRAM scratch, and re-read with the DMA xbar transpose so the
    contraction dim lands on SBUF partitions. W_c^T stays resident; W_in^T /
    W_out^T stream through small tile pools.
  - context: staged fp32 loads -> DVE bf16 cast -> DRAM scratch (natural),
    then one fat xbar-transpose read per d-tile gives ctx^T [d=128, s=2048].
  - source^T psum tiles are [e=128, s=512]; (target + b_c) enters as the
    per-partition bias of the tanh activation; attn^T tiles are bf16.
  - scores accumulate over e-tiles as four [1, 512] rows packed at partition
    bases 0/32/64/96 of ONE psum bank via explicit tile_position.
  - softmax runs on a gathered [1, 2048] row without max-subtraction
    (|scores| <= sum|w_v| ~ 16, safe in fp32); attn_w row DMAs straight out,
    and a bf16 copy bounces through DRAM to come back column-major for the
    weighted sum.
  - the output GEMM uses x^T/weighted^T columns (PE transposes) against
    streamed W_out^T tiles.
"""

import numpy as np

B, S, D = 32, 2048, 1024
N_CORES = 8
BL = B // N_CORES  # batch rows per core
ST = S // 128      # s-tiles of 128 per batch
SC = S // 512      # s-chunks of 512 per batch
DT = D // 128      # d/e tiles of 128
KT = 2 * D // 128  # k tiles for the output GEMM

_CACHE = {}


def _build_nc():
    from contextlib import ExitStack

    import concourse.bacc as bacc
    import concourse.bass as bass
    import concourse.tile as tile
    from concourse import mybir
    from concourse.masks import make_identity

    fp32 = mybir.dt.float32
    bf16 = mybir.dt.bfloat16
    AF = mybir.ActivationFunctionType

    nc = bacc.Bacc(
        "TRN2",
        target_bir_lowering=False,
        debug=False,
        enable_asserts=False,
        num_devices=N_CORES,
    )

    x_d = nc.dram_tensor("x", [BL, D], fp32, kind="ExternalInput")
    ctx_d = nc.dram_tensor("context", [BL, S, D], fp32, kind="ExternalInput")
    win_d = nc.dram_tensor("W_in", [D, D], fp32, kind="ExternalInput")
    wc_d = nc.dram_tensor("W_c", [D, D], fp32, kind="ExternalInput")
    bc_d = nc.dram_tensor("b_c", [D], fp32, kind="ExternalInput")
    wv_d = nc.dram_tensor("w_v", [D], fp32, kind="ExternalInput")
    wout_d = nc.dram_tensor("W_out", [D, 2 * D], fp32, kind="ExternalInput")
    h_d = nc.dram_tensor("h_tilde", [BL, D], fp32, kind="ExternalOutput")
    aw_d = nc.dram_tensor("attn_w", [BL, S], fp32, kind="ExternalOutput")

    with tile.TileContext(nc) as tc, ExitStack() as ctx:
        consts = ctx.enter_context(tc.tile_pool(name="consts", bufs=1))
        weights = ctx.enter_context(tc.tile_pool(name="weights", bufs=1))
        dram = ctx.enter_context(tc.tile_pool(name="dram", bufs=1, space="DRAM"))
        stagep = ctx.enter_context(tc.tile_pool(name="stagep", bufs=4))
        natp = ctx.enter_context(tc.tile_pool(name="natp", bufs=3))
        natwp = ctx.enter_context(tc.tile_pool(name="natwp", bufs=4))
        ctxTp = ctx.enter_context(tc.tile_pool(name="ctxTp", bufs=18))
        attnp = ctx.enter_context(tc.tile_pool(name="attnp", bufs=8))
        woutTp = ctx.enter_context(tc.tile_pool(name="woutTp", bufs=4))
        batchp = ctx.enter_context(tc.tile_pool(name="batchp", bufs=2))
        psum1 = ctx.enter_context(tc.tile_pool(name="psum1", bufs=4, space="PSUM"))
        psum_sc = ctx.enter_context(tc.tile_pool(name="psum_sc", bufs=2, space="PSUM"))
        psum_w = ctx.enter_context(tc.tile_pool(name="psum_w", bufs=2, space="PSUM"))

        # ---------------- constants ----------------
        ident = consts.tile([128, 128], fp32)
        make_identity(nc, ident)
        ident_bf = consts.tile([BL, BL], bf16)
        make_identity(nc, ident_bf)

        bc_rows = consts.tile([BL, D], fp32)
        bc_ap = bc_d[:]
        nc.gpsimd.dma_start(
            out=bc_rows,
            in_=bass.AP(
                tensor=bc_ap.tensor, offset=bc_ap.offset, ap=[[0, BL], *bc_ap.ap]
            ),
        )
        x_rows = consts.tile([BL, D], fp32)
        nc.sync.dma_start(out=x_rows, in_=x_d[:])

        wcT = weights.tile([128, DT, D], bf16)        # [d_in, d_tile, e] resident
        wv_cols = weights.tile([128, DT], bf16)       # w_v columns per e-tile
        bias_cols = weights.tile([128, DT, BL], fp32)  # (target + b_c)^T columns
        xT_bf = consts.tile([128, DT, BL], bf16)
        weighted_flat = consts.tile([1, BL * D], bf16)  # on partition 0
        w_rows4 = consts.tile([BL, D], bf16)
        h_rows = consts.tile([BL, D], fp32)
        catT_bf = consts.tile([128, DT, BL], bf16)

        wout_scr = dram.tile([D, 2 * D], bf16, tag="woutscr")

        # ---- context natural production for ALL batches, emitted first so
        # the load/cast/write pipeline starts at t=0 alongside weight setup
        ctxscrs = []
        for b in range(BL):
            ctxscr = dram.tile([S, D], bf16, tag="ctxscr", bufs=BL, name=f"ctxscr{b}")
            ctxscrs.append(ctxscr)
            for st in range(ST):
                stg = stagep.tile([128, D], fp32, tag="stg", name=f"stg{b}_{st}")
                nc.sync.dma_start(
                    out=stg, in_=ctx_d[b, st * 128 : (st + 1) * 128, :]
                )
                natb = natp.tile([128, D], bf16, tag="natb", name=f"natb{b}_{st}")
                nc.vector.tensor_copy(natb, stg)
                nc.sync.dma_start(
                    out=ctxscr[st * 128 : (st + 1) * 128, :], in_=natb
                )

        with tc.tile_pool(name="setup", bufs=1) as setupp:
            def cast_to_scratch(src, scr, col0, n_cols):
                """scr[:, col0:col0+n_cols] <- bf16(src[:, col0:col0+n_cols])
                via staged HWDGE loads + DVE casts (row tiles of 128)."""
                for t in range(D // 128):
                    stg = stagep.tile([128, n_cols], fp32, tag="stg")
                    nc.sync.dma_start(
                        out=stg,
                        in_=src[t * 128 : (t + 1) * 128, col0 : col0 + n_cols],
                    )
                    natb = natp.tile([128, n_cols], bf16, tag="natb")
                    nc.vector.tensor_copy(natb, stg)
                    nc.sync.dma_start(
                        out=scr[t * 128 : (t + 1) * 128, col0 : col0 + n_cols],
                        in_=natb,
                    )

            wc_scr = dram.tile([D, D], bf16, tag="wcscr")
            cast_to_scratch(wc_d, wc_scr, 0, D)
            for t in range(DT):
                nc.scalar.dma_start_transpose(
                    wcT[:, t, :], wc_scr[0:D, t * 128 : (t + 1) * 128]
                )

            # w_v columns: cast to bf16 scratch, xbar-read as [128, DT]
            wv_stg = setupp.tile([1, D], fp32, tag="wvstg")
            nc.sync.dma_start(out=wv_stg, in_=wv_d[:].rearrange("(a d) -> a d", a=1))
            wv_bf = setupp.tile([1, D], bf16, tag="wvbf")
            nc.vector.tensor_copy(wv_bf, wv_stg)
            wv_scr = dram.tile([DT, 128], bf16, tag="wvscr")
            nc.sync.dma_start(out=wv_scr, in_=wv_bf)
            nc.scalar.dma_start_transpose(wv_cols, wv_scr[:])

            win_scr = dram.tile([D, D], bf16, tag="winscr")
            cast_to_scratch(win_d, win_scr, 0, D)

            # x^T columns (also the second half of catT)
            for dt in range(DT):
                ps = psum_w.tile([128, BL], fp32, tag="psw", name=f"psxT{dt}")
                nc.tensor.transpose(
                    ps, x_rows[:, dt * 128 : (dt + 1) * 128], ident[:BL, :BL]
                )
                nc.vector.tensor_copy(xT_bf[:, dt, :], ps)

            # target rows + b_c -> bias columns; W_in^T streamed per tile
            bias_f32 = setupp.tile([BL, D], fp32, tag="biasf")
            ps_t = [
                psum_w.tile([BL, 512], fp32, tag="psw", name=f"ps_t{i}")
                for i in range(2)
            ]
            for dt in range(DT):
                winT_t = setupp.tile([128, D], bf16, tag="winTt", bufs=3)
                nc.scalar.dma_start_transpose(
                    winT_t, win_scr[0:D, dt * 128 : (dt + 1) * 128]
                )
                for eh in range(2):
                    nc.tensor.matmul(
                        ps_t[eh],
                        lhsT=xT_bf[:, dt, :],
                        rhs=winT_t[:, eh * 512 : (eh + 1) * 512],
                        start=(dt == 0),
                        stop=(dt == DT - 1),
                    )
            for eh in range(2):
                nc.vector.tensor_add(
                    bias_f32[:, eh * 512 : (eh + 1) * 512],
                    ps_t[eh],
                    bc_rows[:, eh * 512 : (eh + 1) * 512],
                )
            for et in range(DT):
                ps = psum_w.tile([128, BL], fp32, tag="psw", name=f"psbc{et}")
                nc.tensor.transpose(
                    ps, bias_f32[:, et * 128 : (et + 1) * 128], ident[:BL, :BL]
                )
                nc.vector.tensor_copy(bias_cols[:, et, :], ps)

        # ---------------- main batch loop ----------------
        def emit_weighted(b, aw_cols, ctxscr):
            """attn_w^T @ context for batch b; emitted one batch late so the
            softmax chain never blocks the PE queue ahead of ready pass-1
            matmuls of the next batch."""
            ps_ws = [
                psum_w.tile([1, 512], fp32, tag="psw", name=f"ps_w{b}_{i}")
                for i in range(2)
            ]
            for st in range(ST):
                natw = natwp.tile([128, D], bf16, tag="natw", name=f"natw{b}_{st}")
                nc.sync.dma_start(
                    out=natw, in_=ctxscr[st * 128 : (st + 1) * 128, :]
                )
                for dh in range(2):
                    nc.tensor.matmul(
                        ps_ws[dh],
                        lhsT=aw_cols[:, st : st + 1],
                        rhs=natw[:, dh * 512 : (dh + 1) * 512],
                        start=(st == 0),
                        stop=(st == ST - 1),
                    )
            for dh in range(2):
                nc.vector.tensor_copy(
                    weighted_flat[0:1, b * D + dh * 512 : b * D + (dh + 1) * 512],
                    ps_ws[dh],
                )

            # W_out bf16 scratch, emitted last in setup (only the output GEMM
            # reads it); loads ride the gpsimd queue to keep sync/scalar free
            cast_to_scratch(wout_d, wout_scr, 0, D)
            cast_to_scratch(wout_d, wout_scr, D, D)

        pending_weighted = None
        for b in range(BL):
            ctxscr = ctxscrs[b]
            # transposed context, one fat xbar read per d-tile
            ctxTs = []
            for dt in range(DT):
                t = ctxTp.tile([128, S], bf16, tag="ctxT")
                nc.scalar.dma_start_transpose(
                    t, ctxscr[0:S, dt * 128 : (dt + 1) * 128]
                )
                ctxTs.append(t)

            # source^T + tanh + scores
            ps_sc = psum_sc.tile([128, 512], fp32, tag="psc")
            for et in range(DT):
                pss = [
                    psum1.tile([128, 512], fp32, tag="ps1", name=f"ps1_{et}_{sc}")
                    for sc in range(SC)
                ]
                for dt in range(DT):
                    lw = wcT[:, dt, et * 128 : (et + 1) * 128]
                    for sc in range(SC):
                        nc.tensor.matmul(
                            pss[sc],
                            lhsT=lw,
                            rhs=ctxTs[dt][:, sc * 512 : (sc + 1) * 512],
                            start=(dt == 0),
                            stop=(dt == DT - 1),
                        )
                attns = []
                for sc in range(SC):
                    at = attnp.tile([128, 512], bf16, tag="attn")
                    nc.scalar.activation(
                        at, pss[sc], AF.Tanh, bias=bias_cols[:, et, b : b + 1]
                    )
                    attns.append(at)
                for sc in range(SC):
                    nc.tensor.matmul(
                        ps_sc[32 * sc : 32 * sc + 1, :],
                        lhsT=wv_cols[:, et : et + 1],
                        rhs=attns[sc],
                        start=(et == 0),
                        stop=(et == DT - 1),
                        tile_position=(0, 32 * sc),
                    )

            # gather scores into one row
            sc_sb = batchp.tile([128, 512], fp32, tag="scsb")
            for sc in range(SC):
                nc.vector.tensor_copy(
                    sc_sb[32 * sc : 32 * sc + 1, :],
                    ps_sc[32 * sc : 32 * sc + 1, :],
                )
            scores_row = batchp.tile([1, S], fp32, tag="srow")
            for sc in range(SC):
                nc.sync.dma_start(
                    out=scores_row[0:1, sc * 512 : (sc + 1) * 512],
                    in_=sc_sb[32 * sc : 32 * sc + 1, :],
                )

            # softmax on the row, in place (no max subtraction: safe in fp32)
            l_acc = batchp.tile([1, 1], fp32, tag="lacc")
            nc.scalar.activation(scores_row, scores_row, AF.Exp, accum_out=l_acc)
            rl = batchp.tile([1, 1], fp32, tag="rl")
            nc.vector.reciprocal(rl, l_acc)
            nc.vector.tensor_scalar_mul(scores_row, scores_row, rl)
            nc.sync.dma_start(
                out=aw_d[b].rearrange("(a s) -> a s", a=1), in_=scores_row
            )
            aw_rbf = batchp.tile([1, S], bf16, tag="awrbf")
            nc.vector.tensor_copy(aw_rbf, scores_row)
            awscr = dram.tile([ST, 128], bf16, tag="awscr", bufs=2)
            nc.sync.dma_start(out=awscr, in_=aw_rbf)
            aw_cols = batchp.tile([128, ST], bf16, tag="awcols")
            nc.scalar.dma_start_transpose(aw_cols, awscr[:])

            if pending_weighted is not None:
                emit_weighted(*pending_weighted)
            pending_weighted = (b, aw_cols, ctxscr)
        emit_weighted(*pending_weighted)

        # ---------------- output GEMM ----------------
        wf_scr = dram.tile([1, BL * D], bf16, tag="wfscr")
        nc.sync.dma_start(out=wf_scr, in_=weighted_flat)
        nc.sync.dma_start(
            out=w_rows4,
            in_=wf_scr[:].rearrange("a (b d) -> b (a d)", b=BL),
        )
        for dt in range(DT):
            ps = psum_w.tile([128, BL], bf16, tag="psw", name=f"pswT{dt}")
            nc.tensor.transpose(
                ps, w_rows4[:, dt * 128 : (dt + 1) * 128], ident_bf
            )
            nc.vector.tensor_copy(catT_bf[:, dt, :], ps)
        ps_h = [
            psum_w.tile([BL, 512], fp32, tag="psw", name=f"ps_h{i}")
            for i in range(2)
        ]
        for kt in range(KT):
            woutT_t = woutTp.tile([128, D], bf16, tag="woutTt")
            nc.scalar.dma_start_transpose(
                woutT_t, wout_scr[0:D, kt * 128 : (kt + 1) * 128]
            )
            lhsT = catT_bf[:, kt, :] if kt < DT else xT_bf[:, kt - DT, :]
            for oh in range(2):
                nc.tensor.matmul(
                    ps_h[oh],
                    lhsT=lhsT,
                    rhs=woutT_t[:, oh * 512 : (oh + 1) * 512],
                    start=(kt == 0),
                    stop=(kt == KT - 1),
                )
        for oh in range(2):
            nc.scalar.activation(
                h_rows[:, oh * 512 : (oh + 1) * 512], ps_h[oh], AF.Tanh
            )
        nc.sync.dma_start(out=h_d[:], in_=h_rows)

    nc.compile()
    return nc


def get_nc():
    if "nc" not in _CACHE:
        _CACHE["nc"] = _build_nc()
    return _CACHE["nc"]


def _make_in_maps(inputs):
    x = np.ascontiguousarray(np.asarray(inputs["x"], dtype=np.float32))
    context = np.ascontiguousarray(np.asarray(inputs["context"], dtype=np.float32))
    weights = {
        k: np.ascontiguousarray(np.asarray(inputs[k], dtype=np.float32))
        for k in ("W_in", "W_c", "b_c", "w_v", "W_out")
    }
    return [
        {
            "x": x[i * BL : (i + 1) * BL],
            "context": context[i * BL : (i + 1) * BL],
            **weights,
        }
        for i in range(N_CORES)
    ]


def kernel(x, context, W_in, W_c, b_c, w_v, W_out):
    from concourse.bass_utils import run_bass_kernel_spmd

    nc = get_nc()
    in_maps = _make_in_maps(
        dict(x=x, context=context, W_in=W_in, W_c=W_c, b_c=b_c, w_v=w_v, W_out=W_out)
    )
    res = run_bass_kernel_spmd(nc, in_maps, list(range(N_CORES)))
    h = np.concatenate([r["h_tilde"] for r in res.results], axis=0)
    aw = np.concatenate([r["attn_w"] for r in res.results], axis=0)
    return h, aw


if __name__ == "__main__":
    import reference as R

    inputs = {k: np.asarray(v) for k, v in R.setup_inputs().items()}
    h, aw = kernel(**inputs)
    print(h.shape, aw.shape, h.dtype, aw.dtype)


# revision 38
# speedup vs baseline: 1.3281x; 1.3281x over previous
"""Trainium2 Bass kernel for BasicAttention (Bahdanau-style additive attention).

Math (per batch row b):
    target  = x @ W_in.T                                   [B, D]
    source  = context @ W_c.T + b_c                        [B, S, D]
    attn    = tanh(target[:, None, :] + source)            [B, S, D]
    scores  = attn @ w_v                                   [B, S]
    attn_w  = softmax(scores, axis=S)                      [B, S]
    weighted= attn_w @ context                             [B, D]
    h_tilde = tanh(concat([weighted, x], -1) @ W_out.T)    [B, D]

Sharding: data-parallel over batch. 8 cores x 4 batch rows, full weights
replicated per core, no collectives. Compute in bf16 (fp32 PSUM accumulation);
~0.2-0.5% of output scale vs the fp32 reference.

Per-core dataflow (all stationary matmul operands are SBUF-resident so their
LDWEIGHTS carry no semaphore waits and the PE's reorder window can pull them
ahead of in-flight matmuls):
  - weights are cast to bf16 via staged HWDGE loads + DVE copies, bounced
    through DRAM scratch, and re-read with the DMA xbar transpose so the
    contraction dim lands on SBUF partitions. W_c^T stays resident; W_in^T /
    W_out^T stream through small tile pools.
  - context: staged fp32 loads -> DVE bf16 cast -> DRAM scratch (natural),
    then one fat xbar-transpose read per d-tile gives ctx^T [d=128, s=2048].
  - source^T psum tiles are [e=128, s=512]; (target + b_c) enters as the
    per-partition bias of the tanh activation; attn^T tiles are bf16.
  - scores accumulate over e-tiles as four [1, 512] rows packed at partition
    bases 0/32/64/96 of ONE psum bank via explicit tile_position.
  - softmax runs on a gathered [1, 2048] row without max-subtraction
    (|scores| <= sum|w_v| ~ 16, safe in fp32); attn_w row DMAs straight out,
    and a bf16 copy bounces through DRAM to come back column-major for the
    weighted sum.
  - the output GEMM uses x^T/weighted^T columns (PE transposes) against
    streamed W_out^T tiles.
"""

import numpy as np

B, S, D = 32, 2048, 1024
N_CORES = 8
BL = B // N_CORES  # batch rows per core
ST = S // 128      # s-tiles of 128 per batch
SC = S // 512      # s-chunks of 512 per batch
DT = D // 128      # d/e tiles of 128
KT = 2 * D // 128  # k tiles for the output GEMM

_CACHE = {}


def _build_nc():
    from contextlib import ExitStack

    import concourse.bacc as bacc
    import concourse.bass as bass
    import concourse.tile as tile
    from concourse import mybir
    from concourse.masks import make_identity

    fp32 = mybir.dt.float32
    bf16 = mybir.dt.bfloat16
    AF = mybir.ActivationFunctionType

    nc = bacc.Bacc(
        "TRN2",
        target_bir_lowering=False,
        debug=False,
        enable_asserts=False,
        num_devices=N_CORES,
    )

    x_d = nc.dram_tensor("x", [BL, D], fp32, kind="ExternalInput")
    ctx_d = nc.dram_tensor("context", [BL, S, D], fp32, kind="ExternalInput")
    win_d = nc.dram_tensor("W_in", [D, D], fp32, kind="ExternalInput")
    wc_d = nc.dram_tensor("W_c", [D, D], fp32, kind="ExternalInput")
    bc_d = nc.dram_tensor("b_c", [D], fp32, kind="ExternalInput")
    wv_d = nc.dram_tensor("w_v", [D], fp32, kind="ExternalInput")
    wout_d = nc.dram_tensor("W_out", [D, 2 * D], fp32, kind="ExternalInput")
    h_d = nc.dram_tensor("h_tilde", [BL, D], fp32, kind="ExternalOutput")
    aw_d = nc.dram_tensor("attn_w", [BL, S], fp32, kind="ExternalOutput")

    with tile.TileContext(nc) as tc, ExitStack() as ctx:
        consts = ctx.enter_context(tc.tile_pool(name="consts", bufs=1))
        weights = ctx.enter_context(tc.tile_pool(name="weights", bufs=1))
        dram = ctx.enter_context(tc.tile_pool(name="dram", bufs=1, space="DRAM"))
        stagep = ctx.enter_context(tc.tile_pool(name="stagep", bufs=4))
        natp = ctx.enter_context(tc.tile_pool(name="natp", bufs=3))
        natwp = ctx.enter_context(tc.tile_pool(name="natwp", bufs=4))
        ctxTp = ctx.enter_context(tc.tile_pool(name="ctxTp", bufs=18))
        attnp = ctx.enter_context(tc.tile_pool(name="attnp", bufs=8))
        woutTp = ctx.enter_context(tc.tile_pool(name="woutTp", bufs=4))
        batchp = ctx.enter_context(tc.tile_pool(name="batchp", bufs=2))
        psum1 = ctx.enter_context(tc.tile_pool(name="psum1", bufs=4, space="PSUM"))
        psum_sc = ctx.enter_context(tc.tile_pool(name="psum_sc", bufs=2, space="PSUM"))
        psum_w = ctx.enter_context(tc.tile_pool(name="psum_w", bufs=2, space="PSUM"))

        # ---------------- constants ----------------
        ident = consts.tile([128, 128], fp32)
        make_identity(nc, ident)
        ident_bf = consts.tile([BL, BL], bf16)
        make_identity(nc, ident_bf)

        bc_rows = consts.tile([BL, D], fp32)
        bc_ap = bc_d[:]
        nc.gpsimd.dma_start(
            out=bc_rows,
            in_=bass.AP(
                tensor=bc_ap.tensor, offset=bc_ap.offset, ap=[[0, BL], *bc_ap.ap]
            ),
        )
        x_rows = consts.tile([BL, D], fp32)
        nc.sync.dma_start(out=x_rows, in_=x_d[:])

        wcT = weights.tile([128, DT, D], bf16)        # [d_in, d_tile, e] resident
        wv_cols = weights.tile([128, DT], bf16)       # w_v columns per e-tile
        bias_cols = weights.tile([128, DT, BL], fp32)  # (target + b_c)^T columns
        xT_bf = consts.tile([128, DT, BL], bf16)
        weighted_flat = consts.tile([1, BL * D], bf16)  # on partition 0
        w_rows4 = consts.tile([BL, D], bf16)
        h_rows = consts.tile([BL, D], fp32)
        catT_bf = consts.tile([128, DT, BL], bf16)

        wout_scr = dram.tile([D, 2 * D], bf16, tag="woutscr")

        # ---- context natural production (load fp32, DVE-cast, write bf16
        # scratch); batch 0 is emitted before weight setup so its pipeline
        # starts immediately, later batches interleave with compute
        ctxscrs = [
            dram.tile([S, D], bf16, tag="ctxscr", bufs=BL, name=f"ctxscr{b}")
            for b in range(BL)
        ]

        def emit_production(b):
            ctxscr = ctxscrs[b]
            for st in range(ST):
                stg = stagep.tile([128, D], fp32, tag="stg", name=f"stg{b}_{st}")
                nc.sync.dma_start(
                    out=stg, in_=ctx_d[b, st * 128 : (st + 1) * 128, :]
                )
                natb = natp.tile([128, D], bf16, tag="natb", name=f"natb{b}_{st}")
                nc.vector.tensor_copy(natb, stg)
                nc.sync.dma_start(
                    out=ctxscr[st * 128 : (st + 1) * 128, :], in_=natb
                )

        emit_production(0)

        with tc.tile_pool(name="setup", bufs=1) as setupp:
            def cast_to_scratch(src, scr, col0, n_cols):
                """scr[:, col0:col0+n_cols] <- bf16(src[:, col0:col0+n_cols])
                via staged HWDGE loads + DVE casts (row tiles of 128)."""
                for t in range(D // 128):
                    stg = stagep.tile([128, n_cols], fp32, tag="stg")
                    nc.sync.dma_start(
                        out=stg,
                        in_=src[t * 128 : (t + 1) * 128, col0 : col0 + n_cols],
                    )
                    natb = natp.tile([128, n_cols], bf16, tag="natb")
                    nc.vector.tensor_copy(natb, stg)
                    nc.sync.dma_start(
                        out=scr[t * 128 : (t + 1) * 128, col0 : col0 + n_cols],
                        in_=natb,
                    )

            wc_scr = dram.tile([D, D], bf16, tag="wcscr")
            cast_to_scratch(wc_d, wc_scr, 0, D)
            for t in range(DT):
                nc.scalar.dma_start_transpose(
                    wcT[:, t, :], wc_scr[0:D, t * 128 : (t + 1) * 128]
                )

            # w_v columns: cast to bf16 scratch, xbar-read as [128, DT]
            wv_stg = setupp.tile([1, D], fp32, tag="wvstg")
            nc.sync.dma_start(out=wv_stg, in_=wv_d[:].rearrange("(a d) -> a d", a=1))
            wv_bf = setupp.tile([1, D], bf16, tag="wvbf")
            nc.vector.tensor_copy(wv_bf, wv_stg)
            wv_scr = dram.tile([DT, 128], bf16, tag="wvscr")
            nc.sync.dma_start(out=wv_scr, in_=wv_bf)
            nc.scalar.dma_start_transpose(wv_cols, wv_scr[:])

            win_scr = dram.tile([D, D], bf16, tag="winscr")
            cast_to_scratch(win_d, win_scr, 0, D)

            # x^T columns (also the second half of catT)
            for dt in range(DT):
                ps = psum_w.tile([128, BL], fp32, tag="psw", name=f"psxT{dt}")
                nc.tensor.transpose(
                    ps, x_rows[:, dt * 128 : (dt + 1) * 128], ident[:BL, :BL]
                )
                nc.vector.tensor_copy(xT_bf[:, dt, :], ps)

            # target rows + b_c -> bias columns; W_in^T streamed per tile
            bias_f32 = setupp.tile([BL, D], fp32, tag="biasf")
            ps_t = [
                psum_w.tile([BL, 512], fp32, tag="psw", name=f"ps_t{i}")
                for i in range(2)
            ]
            for dt in range(DT):
                winT_t = setupp.tile([128, D], bf16, tag="winTt", bufs=3)
                nc.scalar.dma_start_transpose(
                    winT_t, win_scr[0:D, dt * 128 : (dt + 1) * 128]
                )
                for eh in range(2):
                    nc.tensor.matmul(
                        ps_t[eh],
                        lhsT=xT_bf[:, dt, :],
                        rhs=winT_t[:, eh * 512 : (eh + 1) * 512],
                        start=(dt == 0),
                        stop=(dt == DT - 1),
                    )
            for eh in range(2):
                nc.vector.tensor_add(
                    bias_f32[:, eh * 512 : (eh + 1) * 512],
                    ps_t[eh],
                    bc_rows[:, eh * 512 : (eh + 1) * 512],
                )
            for et in range(DT):
                ps = psum_w.tile([128, BL], fp32, tag="psw", name=f"psbc{et}")
                nc.tensor.transpose(
                    ps, bias_f32[:, et * 128 : (et + 1) * 128], ident[:BL, :BL]
                )
                nc.vector.tensor_copy(bias_cols[:, et, :], ps)

        # ---------------- main batch loop ----------------
        def emit_weighted(b, aw_cols, ctxscr):
            """attn_w^T @ context for batch b; emitted one batch late so the
            softmax chain never blocks the PE queue ahead of ready pass-1
            matmuls of the next batch."""
            ps_ws = [
                psum_w.tile([1, 512], fp32, tag="psw", name=f"ps_w{b}_{i}")
                for i in range(2)
            ]
            for st in range(ST):
                natw = natwp.tile([128, D], bf16, tag="natw", name=f"natw{b}_{st}")
                nc.sync.dma_start(
                    out=natw, in_=ctxscr[st * 128 : (st + 1) * 128, :]
                )
                for dh in range(2):
                    nc.tensor.matmul(
                        ps_ws[dh],
                        lhsT=aw_cols[:, st : st + 1],
                        rhs=natw[:, dh * 512 : (dh + 1) * 512],
                        start=(st == 0),
                        stop=(st == ST - 1),
                    )
            for dh in range(2):
                nc.vector.tensor_copy(
                    weighted_flat[0:1, b * D + dh * 512 : b * D + (dh + 1) * 512],
                    ps_ws[dh],
                )

            _cts = cast_to_scratch

        def cast_to_scratch_wout():
            # W_out bf16 scratch, emitted mid-kernel (only the output GEMM
            # reads it) so it never contends with the startup chain
            for col0 in (0, D):
                for t in range(DT):
                    stg = stagep.tile([128, D], fp32, tag="stg", name=f"wo{col0}_{t}")
                    nc.sync.dma_start(
                        out=stg,
                        in_=wout_d[t * 128 : (t + 1) * 128, col0 : col0 + D],
                    )
                    natb = natp.tile(
                        [128, D], bf16, tag="natb", name=f"wob{col0}_{t}"
                    )
                    nc.vector.tensor_copy(natb, stg)
                    nc.sync.dma_start(
                        out=wout_scr[t * 128 : (t + 1) * 128, col0 : col0 + D],
                        in_=natb,
                    )

        pending_weighted = None
        for b in range(BL):
            ctxscr = ctxscrs[b]
            # transposed context, one fat xbar read per d-tile
            ctxTs = []
            for dt in range(DT):
                t = ctxTp.tile([128, S], bf16, tag="ctxT")
                nc.scalar.dma_start_transpose(
                    t, ctxscr[0:S, dt * 128 : (dt + 1) * 128]
                )
                ctxTs.append(t)

            # source^T + tanh + scores
            ps_sc = psum_sc.tile([128, 512], fp32, tag="psc")
            for et in range(DT):
                pss = [
                    psum1.tile([128, 512], fp32, tag="ps1", name=f"ps1_{et}_{sc}")
                    for sc in range(SC)
                ]
                for dt in range(DT):
                    lw = wcT[:, dt, et * 128 : (et + 1) * 128]
                    for sc in range(SC):
                        nc.tensor.matmul(
                            pss[sc],
                            lhsT=lw,
                            rhs=ctxTs[dt][:, sc * 512 : (sc + 1) * 512],
                            start=(dt == 0),
                            stop=(dt == DT - 1),
                        )
                attns = []
                for sc in range(SC):
                    at = attnp.tile([128, 512], bf16, tag="attn")
                    nc.scalar.activation(
                        at, pss[sc], AF.Tanh, bias=bias_cols[:, et, b : b + 1]
                    )
                    attns.append(at)
                for sc in range(SC):
                    nc.tensor.matmul(
                        ps_sc[32 * sc : 32 * sc + 1, :],
                        lhsT=wv_cols[:, et : et + 1],
                        rhs=attns[sc],
                        start=(et == 0),
                        stop=(et == DT - 1),
                        tile_position=(0, 32 * sc),
                    )

            if b + 1 < BL:
                emit_production(b + 1)
            if b == 1:
                cast_to_scratch_wout()

            # gather scores into one row
            sc_sb = batchp.tile([128, 512], fp32, tag="scsb")
            for sc in range(SC):
                nc.vector.tensor_copy(
                    sc_sb[32 * sc : 32 * sc + 1, :],
                    ps_sc[32 * sc : 32 * sc + 1, :],
                )
            scores_row = batchp.tile([1, S], fp32, tag="srow")
            for sc in range(SC):
                nc.sync.dma_start(
                    out=scores_row[0:1, sc * 512 : (sc + 1) * 512],
                    in_=sc_sb[32 * sc : 32 * sc + 1, :],
                )

            # softmax on the row, in place (no max subtraction: safe in fp32)
            l_acc = batchp.tile([1, 1], fp32, tag="lacc")
            nc.scalar.activation(scores_row, scores_row, AF.Exp, accum_out=l_acc)
            rl = batchp.tile([1, 1], fp32, tag="rl")
            nc.vector.reciprocal(rl, l_acc)
            nc.vector.tensor_scalar_mul(scores_row, scores_row, rl)
            nc.sync.dma_start(
                out=aw_d[b].rearrange("(a s) -> a s", a=1), in_=scores_row
            )
            aw_rbf = batchp.tile([1, S], bf16, tag="awrbf")
            nc.vector.tensor_copy(aw_rbf, scores_row)
            awscr = dram.tile([ST, 128], bf16, tag="awscr", bufs=2)
            nc.sync.dma_start(out=awscr, in_=aw_rbf)
            aw_cols = batchp.tile([128, ST], bf16, tag="awcols")
            nc.scalar.dma_start_transpose(aw_cols, awscr[:])

            if pending_weighted is not None:
                emit_weighted(*pending_weighted)
            pending_weighted = (b, aw_cols, ctxscr)
        emit_weighted(*pending_weighted)

        # ---------------- output GEMM ----------------
        wf_scr = dram.tile([1, BL * D], bf16, tag="wfscr")
        nc.sync.dma_start(out=wf_scr, in_=weighted_flat)
        nc.sync.dma_start(
            out=w_rows4,
            in_=wf_scr[:].rearrange("a (b d) -> b (a d)", b=BL),
        )
        for dt in range(DT):
            ps = psum_w.tile([128, BL], bf16, tag="psw", name=f"pswT{dt}")
            nc.tensor.transpose(
                ps, w_rows4[:, dt * 128 : (dt + 1) * 128], ident_bf
            )
            nc.vector.tensor_copy(catT_bf[:, dt, :], ps)
        ps_h = [
            psum_w.tile([BL, 512], fp32, tag="psw", name=f"ps_h{i}")
            for i in range(2)
        ]
        for kt in range(KT):
            woutT_t = woutTp.tile([128, D], bf16, tag="woutTt")
            nc.scalar.dma_start_transpose(
                woutT_t, wout_scr[0:D, kt * 128 : (kt + 1) * 128]
            )
            lhsT = catT_bf[:, kt, :] if kt < DT else xT_bf[:, kt - DT, :]
            for oh in range(2):
                nc.tensor.matmul(
                    ps_h[oh],
                    lhsT=lhsT,
                    rhs=woutT_t[:, oh * 512 : (oh + 1) * 512],
                    start=(kt == 0),
                    stop=(kt == KT - 1),
                )
        for oh in range(2):
            nc.scalar.activation(
                h_rows[:, oh * 512 : (oh + 1) * 512], ps_h[oh], AF.Tanh
            )
        nc.sync.dma_start(out=h_d[:], in_=h_rows)

    nc.compile()
    return nc


def get_nc():
    if "nc" not in _CACHE:
        _CACHE["nc"] = _build_nc()
    return _CACHE["nc"]


def _make_in_maps(inputs):
    x = np.ascontiguousarray(np.asarray(inputs["x"], dtype=np.float32))
    context = np.ascontiguousarray(np.asarray(inputs["context"], dtype=np.float32))
    weights = {
        k: np.ascontiguousarray(np.asarray(inputs[k], dtype=np.float32))
        for k in ("W_in", "W_c", "b_c", "w_v", "W_out")
    }
    return [
        {
            "x": x[i * BL : (i + 1) * BL],
            "context": context[i * BL : (i + 1) * BL],
            **weights,
        }
        for i in range(N_CORES)
    ]


def kernel(x, context, W_in, W_c, b_c, w_v, W_out):
    from concourse.bass_utils import run_bass_kernel_spmd

    nc = get_nc()
    in_maps = _make_in_maps(
        dict(x=x, context=context, W_in=W_in, W_c=W_c, b_c=b_c, w_v=w_v, W_out=W_out)
    )
    res = run_bass_kernel_spmd(nc, in_maps, list(range(N_CORES)))
    h = np.concatenate([r["h_tilde"] for r in res.results], axis=0)
    aw = np.concatenate([r["attn_w"] for r in res.results], axis=0)
    return h, aw


if __name__ == "__main__":
    import reference as R

    inputs = {k: np.asarray(v) for k, v in R.setup_inputs().items()}
    h, aw = kernel(**inputs)
    print(h.shape, aw.shape, h.dtype, aw.dtype)
